# revision 7
# baseline (speedup 1.0000x reference)
"""Trainium2 Bass kernel for nn_LogisticRegression (multi-hot + mean-embedding
logistic regression over a 50k vocab).

Math: for each row i with tokens x[i, 0:200]:
    logit[i] = b + (1/200) * sum_j s[x_ij] + sum_{unique tokens t in row} Wv[t]
    y[i] = sigmoid(logit[i])
where s = E @ w_emb (one scalar per vocab entry), Wv = W[0, 300:].

Device strategy (8 NeuronCores, SPMD):
  - batch-shard rows: 128 rows per core (partition = row).
  - vocab-shard s: each core computes s for vocab ids [6272c, 6272(c+1))
    via PE matmuls on a host-transposed bf16 E shard (rhs = E^T tiles,
    lhsT = w broadcast to 128 columns, PSUM accumulation over 3 e-chunks).
  - a per-vocab fp8 table is built on device: entry e (=2 vocab ids) packs
    (ct[2e], s[2e], ct[2e+1], s[2e+1]) where ct = s/200 + Wv, scaled for
    fp8 range. Shards are AllGathered (12.5KB/core) then broadcast to all
    128 partitions (100KB/partition).
  - ONE gpsimd ap_gather (3200 idx/core, all 8 Q7 cores in parallel)
    looks up 4 fp8 lanes per token from the SBUF-resident table. This
    replaces per-token DMA-gather descriptor generation (the baseline
    bottleneck: ~8ns/token serialized on 2 Q7 cores).
  - dedup: rows are sorted (Max8+MatchReplace); first-occurrence tokens
    read the combined ct lane, duplicates read the s-only lane. The
    lane/dedup/scale selection is folded into one mask tensor, so a
    single fused multiply-reduce produces the logits. Exact for any
    duplicate count.
"""
import sys

sys.path.insert(0, "/opt/trn_rl_repo")

import numpy as np

import concourse.bass as bass
import concourse.bacc as bacc
import concourse.mybir as mybir
import concourse.tile as tile

N_CORES = 8
P = 128             # rows per core
L = 200             # tokens per row
VOCAB = 50000
EMB = 300
SHARD = 6272        # vocab ids per core (identity layout, zero-padded)
VTOT = SHARD * N_CORES   # 50176
NE = VTOT // 2      # ap_gather entries (2 vocab ids each) = 25088
TBYTES = NE * 4     # fp8 table bytes per partition = 100352
NCH = 13            # s chunks: 12x512 + 1x128 = 6272
SC_CT = 256.0       # fp8 scale for ct lane
SC_S = 16.0         # fp8 scale for s lane
F32 = mybir.dt.float32
BF16 = mybir.dt.bfloat16
F8 = mybir.dt.float8e4
I32 = mybir.dt.int32
I16 = mybir.dt.int16

_CACHE = {}


def build_nc():
    nc = bacc.Bacc("TRN2", target_bir_lowering=False, debug=True)
    x_in = nc.dram_tensor("x_in", [P, L], I32, kind="ExternalInput")
    eT0 = nc.dram_tensor("eT0", [128, SHARD], BF16, kind="ExternalInput")
    eT1 = nc.dram_tensor("eT1", [128, SHARD], BF16, kind="ExternalInput")
    eT2 = nc.dram_tensor("eT2", [44, SHARD], BF16, kind="ExternalInput")
    wbc_in = nc.dram_tensor("wbc_in", [EMB, 128], BF16, kind="ExternalInput")
    wv_in = nc.dram_tensor("wv_in", [P, 49], F32, kind="ExternalInput")
    bias_in = nc.dram_tensor("bias_in", [1, 1], F32, kind="ExternalInput")
    y_out = nc.dram_tensor("y_out", [P, 1], F32, kind="ExternalOutput")

    s_dram = nc.dram_tensor("s_dram", [1, SHARD], F32)
    ct_shard = nc.dram_tensor("ct_shard", [1, SHARD * 2], F8)
    ct_full = nc.dram_tensor("ct_full", [1, TBYTES], F8, addr_space="Shared")
    g_dram = nc.dram_tensor("g_dram", [P, L * 64], F8)

    eT = [eT0, eT1, eT2]
    with tile.TileContext(nc) as tc:
        with (
            tc.tile_pool(name="sb", bufs=1) as pool,
            tc.tile_pool(name="et", bufs=2) as epool,
            tc.tile_pool(name="ps", bufs=4, space="PSUM") as psum_pool,
        ):
            # ---- prefault the ap_gather ucode library early ----------------
            dum_t = pool.tile([P, 64], F8)
            nc.vector.memset(dum_t[:], 0.0)
            dum_i = pool.tile([P, 1], I16)
            nc.vector.memset(dum_i[:], 0)
            dum_o = pool.tile([P, 64], F8)
            nc.gpsimd.ap_gather(
                out_ap=dum_o[:], in_ap=dum_t[:], idxs_ap=dum_i[:],
                channels=P, num_elems=16, d=4, num_idxs=16,
            )

            # ---- tokens: sort, dedup weights, lane masks, gather idx -------
            x_sb = pool.tile([P, L], I32)
            nc.sync.dma_start(out=x_sb[:], in_=x_in[:])
            xf = pool.tile([P, L], F32)
            nc.vector.tensor_copy(out=xf[:], in_=x_sb[:])
            xs = pool.tile([P, L], F32)
            work = pool.tile([P, L], F32)
            nc.vector.tensor_copy(out=work[:], in_=xf[:])
            for k in range(L // 8):
                nc.vector.max(out=xs[:, 8 * k:8 * k + 8], in_=work[:])
                nc.vector.match_replace(
                    out=work[:],
                    in_to_replace=xs[:, 8 * k:8 * k + 8],
                    in_values=work[:],
                    imm_value=-1.0,
                )
            w1 = pool.tile([P, L], F32)
            nc.vector.memset(w1[:, 0:1], 1.0)
            nc.vector.tensor_tensor(
                out=w1[:, 1:L], in0=xs[:, 1:L], in1=xs[:, 0:L - 1],
                op=mybir.AluOpType.not_equal,
            )
            xi = pool.tile([P, L], I32)
            nc.vector.tensor_copy(out=xi[:], in_=xs[:])
            band = pool.tile([P, L], I32)
            nc.vector.tensor_scalar(
                out=band[:], in0=xi[:], scalar1=1, scalar2=None,
                op0=mybir.AluOpType.bitwise_and,
            )
            bf = pool.tile([P, L], F32)
            nc.vector.tensor_copy(out=bf[:], in_=band[:])
            ei32 = pool.tile([P, L], I32)
            nc.vector.tensor_scalar(
                out=ei32[:], in0=xi[:], scalar1=1, scalar2=None,
                op0=mybir.AluOpType.arith_shift_right,
            )
            idx16 = pool.tile([P, L], I16)
            nc.vector.tensor_copy(out=idx16[:], in_=ei32[:])

            # masks: lane (2q = ct, 2q+1 = s) x dedup weight x fp8 unscale
            t0 = pool.tile([P, L], F32)   # w1 / SC_CT
            nc.vector.tensor_scalar(
                out=t0[:], in0=w1[:], scalar1=1.0 / SC_CT, scalar2=None,
                op0=mybir.AluOpType.mult,
            )
            t1 = pool.tile([P, L], F32)   # (1 - w1) / (SC_S * L)
            nc.vector.tensor_scalar(
                out=t1[:], in0=w1[:], scalar1=-1.0 / (SC_S * L),
                scalar2=1.0 / (SC_S * L),
                op0=mybir.AluOpType.mult, op1=mybir.AluOpType.add,
            )
            bn = pool.tile([P, L], F32)   # 1 - bf
            nc.vector.tensor_scalar(
                out=bn[:], in0=bf[:], scalar1=-1.0, scalar2=1.0,
                op0=mybir.AluOpType.mult, op1=mybir.AluOpType.add,
            )
            mask4 = pool.tile([P, L * 4], F32)
            m4v = mask4[:].rearrange("p (j m) -> p j m", m=4)
            nc.vector.tensor_tensor(
                out=m4v[:, :, 0], in0=t0[:], in1=bn[:],
                op=mybir.AluOpType.mult,
            )
            nc.vector.tensor_tensor(
                out=m4v[:, :, 1], in0=t1[:], in1=bn[:],
                op=mybir.AluOpType.mult,
            )
            nc.vector.tensor_tensor(
                out=m4v[:, :, 2], in0=t0[:], in1=bf[:],
                op=mybir.AluOpType.mult,
            )
            nc.vector.tensor_tensor(
                out=m4v[:, :, 3], in0=t1[:], in1=bf[:],
                op=mybir.AluOpType.mult,
            )

            # ---- s shard via PE: s = E_shard @ w_emb -----------------------
            wbc = []
            for e, rows in enumerate((128, 128, 44)):
                wt = pool.tile([rows, 128], BF16, tag=f"wbc{e}")
                nc.sync.dma_start(
                    out=wt[:], in_=wbc_in[128 * e:128 * e + rows, :]
                )
                wbc.append(wt)
            s1 = pool.tile([1, SHARD], F32)
            # groups of 3 chunks (1536 cols); last group = 1 chunk of 128
            groups = [(g * 1536, (512, 512, 512)) for g in range(4)]
            groups.append((6144, (128,)))
            for base, chunks in groups:
                width = sum(chunks)
                ets = []
                for e, rows in enumerate((128, 128, 44)):
                    et = epool.tile([rows, width], BF16, tag=f"et{e}")
                    nc.sync.dma_start(
                        out=et[:], in_=eT[e][:, base:base + width]
                    )
                    ets.append(et)
                off = 0
                for w in chunks:
                    ps = psum_pool.tile([128, w], F32)
                    for e in range(3):
                        nc.tensor.matmul(
                            ps[:],
                            wbc[e][:],
                            ets[e][:, off:off + w],
                            start=(e == 0), stop=(e == 2),
                        )
                    nc.scalar.copy(
                        out=s1[0:1, base + off:base + off + w], in_=ps[0:1, :]
                    )
                    off += w

            # ---- build fp8 table shard, AllGather, broadcast ---------------
            nc.sync.dma_start(out=s_dram[0:1, :], in_=s1[:])
            s128 = pool.tile([P, 49], F32)
            nc.sync.dma_start(
                out=s128[:], in_=s_dram[0].rearrange("(p g) -> p g", p=P)
            )
            wv128 = pool.tile([P, 49], F32)
            nc.sync.dma_start(out=wv128[:], in_=wv_in[:])
            ctsh = pool.tile([P, 98], F8)
            cv = ctsh[:].rearrange("p (g t) -> p g t", t=2)
            nc.vector.scalar_tensor_tensor(
                out=cv[:, :, 0], in0=s128[:], scalar=SC_CT / L, in1=wv128[:],
                op0=mybir.AluOpType.mult, op1=mybir.AluOpType.add,
            )
            nc.vector.tensor_scalar(
                out=cv[:, :, 1], in0=s128[:], scalar1=SC_S / L, scalar2=None,
                op0=mybir.AluOpType.mult,
            )
            nc.sync.dma_start(
                out=ct_shard[0:1, :].rearrange("o (p b) -> (o p) b", p=P),
                in_=ctsh[:],
            )
            nc.gpsimd.collective_compute(
                "AllGather", mybir.AluOpType.bypass,
                replica_groups=[list(range(N_CORES))],
                ins=[ct_shard[0:1, :]],
                outs=[ct_full[0:1, :].rearrange("o (a b) -> (o a) b", a=N_CORES)],
            )
            table = pool.tile([P, TBYTES], F8)
            nc.sync.dma_start(
                out=table[:], in_=ct_full[0:1, :].to_broadcast([P, TBYTES])
            )

            # ---- the gather + fused masked reduce --------------------------
            gout = pool.tile([P, L * 16 * 4], F8)
            nc.gpsimd.ap_gather(
                out_ap=gout[:], in_ap=table[:], idxs_ap=idx16[:],
                channels=P, num_elems=NE, d=4, num_idxs=L * 16,
            )
            # round-trip through DRAM: the per-row window select needs a
            # per-partition offset, affine only in a flat DRAM AP:
            # value (p=16g+w, j, m) at p*12800 + 64j + 4w + m
            #   = g*204800 + w*12804 + 64j + m
            ge = pool.tile([P, L * 4], F8)
            nc.sync.dma_start(out=g_dram[:], in_=gout[:])
            for g in range(8):
                gsrc = g_dram[:].copy()
                gsrc.ap = type(gsrc.ap)(
                    [[12804, 16], [64, L], [1, 4]]
                )
                gsrc.offset = g * 204800
                nc.sync.dma_start(out=ge[16 * g:16 * (g + 1), :], in_=gsrc)
            gef = pool.tile([P, L * 4], F32)
            nc.vector.tensor_copy(out=gef[:], in_=ge[:])
            junk = pool.tile([P, L * 4], F32)
            acc = pool.tile([P, 1], F32)
            nc.vector.tensor_tensor_reduce(
                out=junk[:], in0=gef[:], in1=mask4[:], scale=1.0, scalar=0.0,
                op0=mybir.AluOpType.mult, op1=mybir.AluOpType.add,
                accum_out=acc[:],
            )

            # ---- sigmoid(acc + b) ------------------------------------------
            bb = pool.tile([P, 1], F32)
            nc.sync.dma_start(out=bb[:], in_=bias_in[0:1, :].to_broadcast([P, 1]))
            y_sb = pool.tile([P, 1], F32)
            nc.scalar.activation(
                out=y_sb[:], in_=acc[:],
                func=mybir.ActivationFunctionType.Sigmoid,
                bias=bb[:, 0:1], scale=1.0,
            )
            nc.sync.dma_start(out=y_out[:], in_=y_sb[:])
    nc.compile()
    return nc


def prep_inputs(x, embedding_weight, W, b):
    """Host-side sharding/layout prep (data-independent reformatting)."""
    import ml_dtypes
    x = np.asarray(x)
    E = np.asarray(embedding_weight, dtype=np.float32)
    W = np.asarray(W, dtype=np.float32)
    b = np.asarray(b, dtype=np.float32)
    wemb = W[0, :EMB]
    Wv = W[0, EMB:]

    wbc = np.repeat(wemb[:, None], 128, axis=1).astype(ml_dtypes.bfloat16)
    wv_pad = np.zeros(VTOT, dtype=np.float32)
    wv_pad[:VOCAB] = Wv * SC_CT
    xi = x.astype(np.int32)

    in_maps = []
    for c in range(N_CORES):
        lo = SHARD * c
        hi = min(SHARD * (c + 1), VOCAB)
        esh = np.zeros((SHARD, EMB), dtype=np.float32)
        esh[:hi - lo] = E[lo:hi]
        eshT = np.ascontiguousarray(esh.T).astype(ml_dtypes.bfloat16)
        in_maps.append({
            "x_in": xi[c * P:(c + 1) * P],
            "eT0": eshT[0:128],
            "eT1": eshT[128:256],
            "eT2": eshT[256:300],
            "wbc_in": wbc,
            "wv_in": wv_pad[lo:lo + SHARD].reshape(P, 49),
            "bias_in": b.reshape(1, 1),
        })
    return in_maps


def kernel(**inputs):
    if "nc" not in _CACHE:
        _CACHE["nc"] = build_nc()
    nc = _CACHE["nc"]
    in_maps = prep_inputs(**inputs)
    from concourse.bass_utils import run_bass_kernel_spmd
    r = run_bass_kernel_spmd(nc, in_maps, list(range(N_CORES)))
    y = np.concatenate([r.results[c]["y_out"] for c in range(N_CORES)], axis=0)
    return y.astype(np.float32)
